# revision 6
# baseline (speedup 1.0000x reference)
"""CTBG circuit kernel for Trainium2, data-parallel over batch on 8 NeuronCores.

Algebraic collapse: the reference has no nonlinearity before w1, and gpi_out
feeds only h1, so the masked layers fold into a single effective weight:

  gpe_out = x @ MWgpe + gpe_b            MWgpe = gpe_w * gpe_mask.T
  gpi_out = [x, gpe_out] @ MWgpi + gpi_b MWgpi = gpi_w * gpi_mask.T  (A; B rows)
  h1 = relu(gpi_out @ w1 + b1)
     = relu(x @ Wbig + bbig)
  Wbig = A@w1 + MWgpe @ (B@w1)           [1536, 512]
  bbig = gpe_b @ (B@w1) + gpi_b @ w1 + b1
  out  = relu(relu(h1 @ w2 + b2) @ w3 + b3)

Per-batch FLOPs drop 7.75x; the one-time weight precompute (stage1: B@w1,
stage2: Wbig) runs on every core redundantly, streamed against the DMA.

Layout strategy: host pre-transposes x, gpi_w, gpe_w (pure data movement, no
arithmetic) so that every matmul operand lands in its natural PE layout and
the kernel needs ZERO on-device transposes:
  - MWgpi.T = gpi_w.T * gpi_mask (both [1536 gpi-out, 3072 in], natural)
  - MWgpe.T = gpe_w.T * gpe_mask (both [1536 gpe-out, 1536 in], natural)
  - x.T [1536, B] row-tiles are the feature-major rhs directly.
All heavy loads are SWDGE f32->bf16 cast DMAs in kc-major group tiles
[128, K, cols] (one DMA each); DVE multiplies masks in-place; PE does only
matmuls; ScalarE drains PSUM with bias(+relu).
"""

import numpy as np

NCORES = 8
B = 16384
BS = B // NCORES          # 2048 rows per core
BT = 512                  # batch tile (matmul free dim)
NBT = BS // BT            # 4
DIN = 1536                # x features (= Wbig rows)
GO = 1536                 # gpi output dim (stage1 contraction)
GE = 1536                 # gpe output dim (stage2 contraction)
NIN = 3072                # gpi input dim (x ++ gpe_out)
H = 512                   # mlp hidden
A = 6                     # action dim

KG = GO // 128            # 12  k-tiles for stage1 (gpi-out)
KE = GE // 128            # 12  k-tiles for stage2 (gpe-out)
KX = DIN // 128           # 12  k-tiles for batch L1 (x features)
KH = H // 128             # 4   k-tiles for L2/L3
MB_ALL = NIN // 128       # 24  m-blocks of W' (in-dim of gpi)
MB_A = DIN // 128         # 12  A-part m-blocks (x features)
MG = 2                    # m-blocks per load group (B part)
MGA = 1                   # m-blocks per A/E group (short stage2 tail)
NGB = (MB_ALL - MB_A) // MG   # 6 B-part groups
NGA = MB_A // MGA             # 12 A-part groups
NGE = (DIN // 128) // MGA     # 12 gpe groups
GW = MG * 128             # 256 cols per B group
GWA = MGA * 128           # 128 cols per A/E group

_CACHE = {}


def _build():
    import concourse.bacc as bacc
    import concourse.tile as tile
    from concourse import mybir

    FP32 = mybir.dt.float32
    BF16 = mybir.dt.bfloat16
    Act = mybir.ActivationFunctionType

    nc = bacc.Bacc(None)

    xt_d = nc.dram_tensor("x_t", [DIN, BS], FP32, kind="ExternalInput")
    giw_d = nc.dram_tensor("gpi_wt", [GO, NIN], FP32, kind="ExternalInput")
    gim_d = nc.dram_tensor("gpi_mask", [GO, NIN], FP32, kind="ExternalInput")
    gew_d = nc.dram_tensor("gpe_wt", [GE, DIN], FP32, kind="ExternalInput")
    gem_d = nc.dram_tensor("gpe_mask", [GE, DIN], FP32, kind="ExternalInput")
    w1_d = nc.dram_tensor("w1", [GO, H], FP32, kind="ExternalInput")
    w2_d = nc.dram_tensor("w2", [H, H], FP32, kind="ExternalInput")
    w3_d = nc.dram_tensor("w3", [H, A], FP32, kind="ExternalInput")
    geb_d = nc.dram_tensor("gpe_b", [GE], FP32, kind="ExternalInput")
    gib_d = nc.dram_tensor("gpi_b", [GO], FP32, kind="ExternalInput")
    b1_d = nc.dram_tensor("b1", [H], FP32, kind="ExternalInput")
    b2_d = nc.dram_tensor("b2", [H], FP32, kind="ExternalInput")
    b3_d = nc.dram_tensor("b3", [A], FP32, kind="ExternalInput")
    o_d = nc.dram_tensor("out", [A, BS], FP32, kind="ExternalOutput")

    giw_r = giw_d.rearrange("(k p) c -> p k c", p=128)   # [128, KG, NIN]
    gim_r = gim_d.rearrange("(k p) c -> p k c", p=128)
    gew_r = gew_d.rearrange("(k p) c -> p k c", p=128)   # [128, KE, DIN]
    gem_r = gem_d.rearrange("(k p) c -> p k c", p=128)
    w1_r = w1_d.rearrange("(k p) c -> p k c", p=128)     # [128, KG, H]
    w2_r = w2_d.rearrange("(k p) c -> p k c", p=128)     # [128, KH, H]
    w3_r = w3_d.rearrange("(k p) c -> p k c", p=128)     # [128, KH, A]
    xt_r = xt_d.rearrange("(k p) c -> p k c", p=128)     # [128, KX, BS]

    with tile.TileContext(nc) as tc:
        with (
            tc.tile_pool(name="wres", bufs=1) as wr,          # persistent
            tc.tile_pool(name="gbpool", bufs=2) as gbp,
            tc.tile_pool(name="aepool", bufs=3) as aep,       # transient B groups
            tc.tile_pool(name="mpool", bufs=3) as mp,         # transient masks
            tc.tile_pool(name="hpool", bufs=1) as hp,         # h1/h2 activations
            tc.tile_pool(name="xpool", bufs=4) as xp,         # x tiles dbl-buffered
            tc.tile_pool(name="opool", bufs=2) as op,         # out staging
            tc.tile_pool(name="pspool", bufs=4, space="PSUM") as psp,
            tc.tile_pool(name="ps6pool", bufs=2, space="PSUM") as ps6p,
            tc.tile_pool(name="psbpool", bufs=1, space="PSUM") as psbp,
        ):
            # ---------- SWDGE cast loads (consumption order) ----------
            # w1 first (stage1 rhs), then B-part gpi groups (stage1 lhsT),
            # then A-part + gpe groups interleaved (stage2), then w2/w3/bias
            # cols, then x batch tiles.
            # bias columns first, on the idle sync/HWDGE queue (f32; their
            # per-element descriptors would stall the SWDGE weight stream)
            gebf = wr.tile([128, KE], FP32, tag="gebf")
            nc.sync.dma_start(out=gebf[:, :],
                              in_=geb_d.rearrange("(k p) -> p k", p=128))
            gibf = wr.tile([128, KG], FP32, tag="gibf")
            nc.sync.dma_start(out=gibf[:, :],
                              in_=gib_d.rearrange("(k p) -> p k", p=128))
            b1c = wr.tile([128, KH], FP32, tag="b1c")
            nc.sync.dma_start(out=b1c[:, :],
                              in_=b1_d.rearrange("(k p) -> p k", p=128))
            b2c = wr.tile([128, KH], FP32, tag="b2c")
            nc.sync.dma_start(out=b2c[:, :],
                              in_=b2_d.rearrange("(k p) -> p k", p=128))
            b3c = wr.tile([A, 1], FP32, tag="b3c")
            nc.sync.dma_start(out=b3c[:, :],
                              in_=b3_d.rearrange("(a one) -> a one", one=1))
            w3f = wr.tile([128, KH, A], FP32, tag="w3f")
            nc.sync.dma_start(out=w3f[:, :, :], in_=w3_r[:, :, :])
            gebc = wr.tile([128, KE], BF16, tag="gebc")
            nc.vector.tensor_copy(gebc[:, :], gebf[:, :])
            gibc = wr.tile([128, KG], BF16, tag="gibc")
            nc.vector.tensor_copy(gibc[:, :], gibf[:, :])
            w3t = wr.tile([128, KH, A], BF16, tag="w3t")
            nc.vector.tensor_copy(w3t[:, :, :], w3f[:, :, :])

            w1t = wr.tile([128, KG, H], BF16, tag="w1t")
            nc.gpsimd.dma_start(out=w1t[:, :, :], in_=w1_r[:, :, :])

            def load_group(w_r, m_r, c0, tag, pool, gw=GW):
                wt = pool.tile([128, KG, gw], BF16, tag=tag, name=f"wt_{tag}")
                mt = mp.tile([128, KG, gw], BF16, tag=f"mask{gw}", name="maskt")
                nc.gpsimd.dma_start(out=wt[:, :, :], in_=w_r[:, :, c0:c0 + gw])
                nc.gpsimd.dma_start(out=mt[:, :, :], in_=m_r[:, :, c0:c0 + gw])
                nc.vector.tensor_mul(wt[:, :, :], wt[:, :, :], mt[:, :, :])
                return wt

            # stage1: W'B[m] = sum_k MWgpiT[k, B-col m].T @ w1[k]  -> wpb
            wpb = [wr.tile([128, H], BF16, tag=f"wpb{m}", name=f"wpb{m}")
                   for m in range(MB_A)]
            for g in range(NGB):
                wt = load_group(giw_r, gim_r, DIN + g * GW, "gB", gbp)
                for j in range(MG):
                    m = g * MG + j
                    ps = psp.tile([128, H], mybir.dt.float32, tag="ps")
                    for k in range(KG):
                        nc.tensor.matmul(ps[:, :],
                                         wt[:, k, j * 128:(j + 1) * 128],
                                         w1t[:, k, :],
                                         start=(k == 0), stop=(k == KG - 1))
                    nc.vector.tensor_copy(wpb[m][:, :], ps[:, :])


            # bbig = gpe_b @ W'B + gpi_b @ w1 + b1   [H] as [128, KH] cols
            psb = psbp.tile([128, KH], mybir.dt.float32, tag="psb")
            for m4 in range(KH):
                n_mm = KG + KE
                i = 0
                for k in range(KE):
                    nc.tensor.matmul(psb[:, m4:m4 + 1],
                                     wpb[k][:, m4 * 128:(m4 + 1) * 128],
                                     gebc[:, k:k + 1],
                                     start=(i == 0), stop=(i == n_mm - 1))
                    i += 1
                for k in range(KG):
                    nc.tensor.matmul(psb[:, m4:m4 + 1],
                                     w1t[:, k, m4 * 128:(m4 + 1) * 128],
                                     gibc[:, k:k + 1],
                                     start=(i == 0), stop=(i == n_mm - 1))
                    i += 1
            bbig = wr.tile([128, KH], FP32, tag="bbig")
            nc.vector.tensor_add(bbig[:, :], psb[:, :], b1c[:, :])

            # mlp weights
            w2t = wr.tile([128, KH, H], BF16, tag="w2t")
            nc.gpsimd.dma_start(out=w2t[:, :, :], in_=w2_r[:, :, :])

            # stage2: Wbig[m] = sum_k MWgpiT[k, A-col m].T @ w1[k]
            #                 + sum_k MWgpeT[k, col m].T @ W'B[k]
            wbig = [wr.tile([128, H], BF16, tag=f"wbig{m}", name=f"wbig{m}")
                    for m in range(KX)]
            for g in range(NGA):
                at = load_group(giw_r, gim_r, g * GWA, "gA", aep, GWA)
                et = load_group(gew_r, gem_r, g * GWA, "gE", aep, GWA)
                m = g
                ps = psp.tile([128, H], mybir.dt.float32, tag="ps")
                n_mm = KG + KE
                i = 0
                for k in range(KG):
                    nc.tensor.matmul(ps[:, :],
                                     at[:, k, 0:128],
                                     w1t[:, k, :],
                                     start=(i == 0), stop=(i == n_mm - 1))
                    i += 1
                for k in range(KE):
                    nc.tensor.matmul(ps[:, :],
                                     et[:, k, 0:128],
                                     wpb[k][:, :],
                                     start=(i == 0), stop=(i == n_mm - 1))
                    i += 1
                nc.vector.tensor_copy(wbig[m][:, :], ps[:, :])

            # ---------- batch loop ----------
            for bt in range(NBT):
                xt = xp.tile([128, KX, BT], BF16, tag="xt")
                nc.gpsimd.dma_start(out=xt[:, :, :],
                                    in_=xt_r[:, :, bt * BT:(bt + 1) * BT])

                h1 = []
                for u in range(KH):
                    ps = psp.tile([128, BT], mybir.dt.float32, tag="ps")
                    for k in range(KX):
                        nc.tensor.matmul(ps[:, :],
                                         wbig[k][:, u * 128:(u + 1) * 128],
                                         xt[:, k, :],
                                         start=(k == 0), stop=(k == KX - 1))
                    t = hp.tile([128, BT], BF16, tag=f"h1_{u}")
                    nc.scalar.activation(t[:, :], ps[:, :], Act.Relu,
                                         bias=bbig[:, u:u + 1])
                    h1.append(t)

                h2 = []
                for m in range(KH):
                    ps = psp.tile([128, BT], mybir.dt.float32, tag="ps")
                    for k in range(KH):
                        nc.tensor.matmul(ps[:, :],
                                         w2t[:, k, m * 128:(m + 1) * 128],
                                         h1[k][:, :],
                                         start=(k == 0), stop=(k == KH - 1))
                    t = hp.tile([128, BT], BF16, tag=f"h2_{m}")
                    nc.scalar.activation(t[:, :], ps[:, :], Act.Relu,
                                         bias=b2c[:, m:m + 1])
                    h2.append(t)

                ps6 = ps6p.tile([A, BT], mybir.dt.float32, tag="ps6")
                for k in range(KH):
                    nc.tensor.matmul(ps6[:, :], w3t[:, k, :], h2[k][:, :],
                                     start=(k == 0), stop=(k == KH - 1))
                osb = op.tile([A, BT], FP32, tag="osb")
                nc.scalar.activation(osb[:, :], ps6[:, :], Act.Relu,
                                     bias=b3c[:, 0:1])
                nc.sync.dma_start(out=o_d[:, bt * BT:(bt + 1) * BT],
                                  in_=osb[:, :])

    nc.finalize()
    return nc


def _get_nc():
    if "nc" not in _CACHE:
        _CACHE["nc"] = _build()
    return _CACHE["nc"]


def _run(inputs, trace=False):
    from concourse.bass_utils import run_bass_kernel_spmd

    nc = _get_nc()
    f32 = np.float32
    asf = lambda a: np.asarray(a, dtype=f32)
    shared = {
        "gpi_wt": np.ascontiguousarray(asf(inputs["gpi_w"]).T),
        "gpi_mask": np.ascontiguousarray(asf(inputs["gpi_mask"])),
        "gpe_wt": np.ascontiguousarray(asf(inputs["gpe_w"]).T),
        "gpe_mask": np.ascontiguousarray(asf(inputs["gpe_mask"])),
        "w1": np.ascontiguousarray(asf(inputs["w1"])),
        "w2": np.ascontiguousarray(asf(inputs["w2"])),
        "w3": np.ascontiguousarray(asf(inputs["w3"])),
        "gpe_b": np.ascontiguousarray(asf(inputs["gpe_b"])),
        "gpi_b": np.ascontiguousarray(asf(inputs["gpi_b"])),
        "b1": np.ascontiguousarray(asf(inputs["b1"])),
        "b2": np.ascontiguousarray(asf(inputs["b2"])),
        "b3": np.ascontiguousarray(asf(inputs["b3"])),
    }
    xT = np.ascontiguousarray(asf(inputs["x"]).T)   # [DIN, B]
    in_maps = [dict(shared, x_t=np.ascontiguousarray(xT[:, c * BS:(c + 1) * BS]))
               for c in range(NCORES)]
    res = run_bass_kernel_spmd(nc, in_maps, list(range(NCORES)), trace=trace)
    out = np.concatenate(
        [np.asarray(res.results[c]["out"]).T for c in range(NCORES)], axis=0)
    return out.astype(f32), res


def kernel(**inputs):
    out, _ = _run(inputs, trace=False)
    return out


# revision 7
# speedup vs baseline: 1.1558x; 1.1558x over previous
"""CTBG circuit kernel for Trainium2, data-parallel over batch on 8 NeuronCores.

Algebraic collapse: the reference has no nonlinearity before w1, and gpi_out
feeds only h1, so the masked layers fold into a single effective weight:

  gpe_out = x @ MWgpe + gpe_b            MWgpe = gpe_w * gpe_mask.T
  gpi_out = [x, gpe_out] @ MWgpi + gpi_b MWgpi = gpi_w * gpi_mask.T  (A; B rows)
  h1 = relu(gpi_out @ w1 + b1)
     = relu(x @ Wbig + bbig)
  Wbig = A@w1 + MWgpe @ (B@w1)           [1536, 512]
  bbig = gpe_b @ (B@w1) + gpi_b @ w1 + b1
  out  = relu(relu(h1 @ w2 + b2) @ w3 + b3)

Per-batch FLOPs drop 7.75x; the one-time weight precompute (stage1: B@w1,
stage2: Wbig) runs on every core redundantly, streamed against the DMA.

Layout strategy: host pre-transposes x, gpi_w, gpe_w (pure data movement, no
arithmetic) so that every matmul operand lands in its natural PE layout and
the kernel needs ZERO on-device transposes:
  - MWgpi.T = gpi_w.T * gpi_mask (both [1536 gpi-out, 3072 in], natural)
  - MWgpe.T = gpe_w.T * gpe_mask (both [1536 gpe-out, 1536 in], natural)
  - x.T [1536, B] row-tiles are the feature-major rhs directly.
All heavy loads are SWDGE f32->bf16 cast DMAs in kc-major group tiles
[128, K, cols] (one DMA each); DVE multiplies masks in-place; PE does only
matmuls; ScalarE drains PSUM with bias(+relu).
"""

import numpy as np

NCORES = 8
B = 16384
BS = B // NCORES          # 2048 rows per core
BT = 512                  # batch tile (matmul free dim)
NBT = BS // BT            # 4
DIN = 1536                # x features (= Wbig rows)
GO = 1536                 # gpi output dim (stage1 contraction)
GE = 1536                 # gpe output dim (stage2 contraction)
NIN = 3072                # gpi input dim (x ++ gpe_out)
H = 512                   # mlp hidden
A = 6                     # action dim

KG = GO // 128            # 12  k-tiles for stage1 (gpi-out)
KE = GE // 128            # 12  k-tiles for stage2 (gpe-out)
KX = DIN // 128           # 12  k-tiles for batch L1 (x features)
KH = H // 128             # 4   k-tiles for L2/L3
MB_ALL = NIN // 128       # 24  m-blocks of W' (in-dim of gpi)
MB_A = DIN // 128         # 12  A-part m-blocks (x features)
MG = 2                    # m-blocks per load group (B part)
MGA = 1                   # m-blocks per A/E group (short stage2 tail)
NGB = (MB_ALL - MB_A) // MG   # 6 B-part groups
NGA = MB_A // MG              # 6 A-part groups
NGE = (DIN // 128) // MG      # 6 gpe groups
GW = MG * 128             # 256 cols per B group
GWA = MGA * 128           # 128 cols per A/E group

_CACHE = {}


def _build():
    import concourse.bacc as bacc
    import concourse.tile as tile
    from concourse import mybir

    FP32 = mybir.dt.float32
    BF16 = mybir.dt.bfloat16
    Act = mybir.ActivationFunctionType

    nc = bacc.Bacc(None)

    xt_d = nc.dram_tensor("x_t", [DIN, BS], FP32, kind="ExternalInput")
    giw_d = nc.dram_tensor("gpi_wt", [GO, NIN], FP32, kind="ExternalInput")
    gim_d = nc.dram_tensor("gpi_mask", [GO, NIN], FP32, kind="ExternalInput")
    gew_d = nc.dram_tensor("gpe_wt", [GE, DIN], FP32, kind="ExternalInput")
    gem_d = nc.dram_tensor("gpe_mask", [GE, DIN], FP32, kind="ExternalInput")
    w1_d = nc.dram_tensor("w1", [GO, H], FP32, kind="ExternalInput")
    w2_d = nc.dram_tensor("w2", [H, H], FP32, kind="ExternalInput")
    w3_d = nc.dram_tensor("w3", [H, A], FP32, kind="ExternalInput")
    geb_d = nc.dram_tensor("gpe_b", [GE], FP32, kind="ExternalInput")
    gib_d = nc.dram_tensor("gpi_b", [GO], FP32, kind="ExternalInput")
    b1_d = nc.dram_tensor("b1", [H], FP32, kind="ExternalInput")
    b2_d = nc.dram_tensor("b2", [H], FP32, kind="ExternalInput")
    b3_d = nc.dram_tensor("b3", [A], FP32, kind="ExternalInput")
    o_d = nc.dram_tensor("out", [A, BS], FP32, kind="ExternalOutput")

    giw_r = giw_d.rearrange("(k p) c -> p k c", p=128)   # [128, KG, NIN]
    gim_r = gim_d.rearrange("(k p) c -> p k c", p=128)
    gew_r = gew_d.rearrange("(k p) c -> p k c", p=128)   # [128, KE, DIN]
    gem_r = gem_d.rearrange("(k p) c -> p k c", p=128)
    w1_r = w1_d.rearrange("(k p) c -> p k c", p=128)     # [128, KG, H]
    w2_r = w2_d.rearrange("(k p) c -> p k c", p=128)     # [128, KH, H]
    w3_r = w3_d.rearrange("(k p) c -> p k c", p=128)     # [128, KH, A]
    xt_r = xt_d.rearrange("(k p) c -> p k c", p=128)     # [128, KX, BS]

    with tile.TileContext(nc) as tc:
        with (
            tc.tile_pool(name="wres", bufs=1) as wr,          # persistent
            tc.tile_pool(name="gbpool", bufs=2) as gbp,       # transient B groups
            tc.tile_pool(name="mpool", bufs=3) as mp,         # transient masks
            tc.tile_pool(name="hpool", bufs=1) as hp,         # h1/h2 activations
            tc.tile_pool(name="xpool", bufs=2) as xp,         # x tiles dbl-buffered
            tc.tile_pool(name="opool", bufs=2) as op,         # out staging
            tc.tile_pool(name="pspool", bufs=4, space="PSUM") as psp,
            tc.tile_pool(name="ps6pool", bufs=2, space="PSUM") as ps6p,
            tc.tile_pool(name="psbpool", bufs=1, space="PSUM") as psbp,
        ):
            # ---------- SWDGE cast loads (consumption order) ----------
            # w1 first (stage1 rhs), then B-part gpi groups (stage1 lhsT),
            # then A-part + gpe groups interleaved (stage2), then w2/w3/bias
            # cols, then x batch tiles.
            # bias columns first, on the idle sync/HWDGE queue (f32; their
            # per-element descriptors would stall the SWDGE weight stream)
            gebf = wr.tile([128, KE], FP32, tag="gebf")
            nc.sync.dma_start(out=gebf[:, :],
                              in_=geb_d.rearrange("(k p) -> p k", p=128))
            gibf = wr.tile([128, KG], FP32, tag="gibf")
            nc.sync.dma_start(out=gibf[:, :],
                              in_=gib_d.rearrange("(k p) -> p k", p=128))
            b1c = wr.tile([128, KH], FP32, tag="b1c")
            nc.sync.dma_start(out=b1c[:, :],
                              in_=b1_d.rearrange("(k p) -> p k", p=128))
            b2c = wr.tile([128, KH], FP32, tag="b2c")
            nc.sync.dma_start(out=b2c[:, :],
                              in_=b2_d.rearrange("(k p) -> p k", p=128))
            b3c = wr.tile([A, 1], FP32, tag="b3c")
            nc.sync.dma_start(out=b3c[:, :],
                              in_=b3_d.rearrange("(a one) -> a one", one=1))
            w3f = wr.tile([128, KH, A], FP32, tag="w3f")
            nc.sync.dma_start(out=w3f[:, :, :], in_=w3_r[:, :, :])
            gebc = wr.tile([128, KE], BF16, tag="gebc")
            nc.vector.tensor_copy(gebc[:, :], gebf[:, :])
            gibc = wr.tile([128, KG], BF16, tag="gibc")
            nc.vector.tensor_copy(gibc[:, :], gibf[:, :])
            w3t = wr.tile([128, KH, A], BF16, tag="w3t")
            nc.vector.tensor_copy(w3t[:, :, :], w3f[:, :, :])

            w1t = wr.tile([128, KG, H], BF16, tag="w1t")
            nc.gpsimd.dma_start(out=w1t[:, :, :], in_=w1_r[:, :, :])

            def load_group(w_r, m_r, c0, tag, pool, gw=GW):
                wt = pool.tile([128, KG, gw], BF16, tag=tag, name=f"wt_{tag}")
                mt = mp.tile([128, KG, gw], BF16, tag=f"mask{gw}", name="maskt")
                nc.gpsimd.dma_start(out=wt[:, :, :], in_=w_r[:, :, c0:c0 + gw])
                nc.gpsimd.dma_start(out=mt[:, :, :], in_=m_r[:, :, c0:c0 + gw])
                nc.vector.tensor_mul(wt[:, :, :], wt[:, :, :], mt[:, :, :])
                return wt

            # stage1: W'B[m] = sum_k MWgpiT[k, B-col m].T @ w1[k]  -> wpb
            wpb = [wr.tile([128, H], BF16, tag=f"wpb{m}", name=f"wpb{m}")
                   for m in range(MB_A)]
            for g in range(NGB):
                wt = load_group(giw_r, gim_r, DIN + g * GW, "gB", gbp)
                for j in range(MG):
                    m = g * MG + j
                    ps = psp.tile([128, H], mybir.dt.float32, tag="ps")
                    for k in range(KG):
                        nc.tensor.matmul(ps[:, :],
                                         wt[:, k, j * 128:(j + 1) * 128],
                                         w1t[:, k, :],
                                         start=(k == 0), stop=(k == KG - 1))
                    nc.vector.tensor_copy(wpb[m][:, :], ps[:, :])


            # bbig = gpe_b @ W'B + gpi_b @ w1 + b1   [H] as [128, KH] cols
            psb = psbp.tile([128, KH], mybir.dt.float32, tag="psb")
            for m4 in range(KH):
                n_mm = KG + KE
                i = 0
                for k in range(KE):
                    nc.tensor.matmul(psb[:, m4:m4 + 1],
                                     wpb[k][:, m4 * 128:(m4 + 1) * 128],
                                     gebc[:, k:k + 1],
                                     start=(i == 0), stop=(i == n_mm - 1))
                    i += 1
                for k in range(KG):
                    nc.tensor.matmul(psb[:, m4:m4 + 1],
                                     w1t[:, k, m4 * 128:(m4 + 1) * 128],
                                     gibc[:, k:k + 1],
                                     start=(i == 0), stop=(i == n_mm - 1))
                    i += 1
            bbig = wr.tile([128, KH], FP32, tag="bbig")
            nc.vector.tensor_add(bbig[:, :], psb[:, :], b1c[:, :])

            # mlp weights
            w2t = wr.tile([128, KH, H], BF16, tag="w2t")
            nc.gpsimd.dma_start(out=w2t[:, :, :], in_=w2_r[:, :, :])

            # stage2: Wbig[m] = sum_k MWgpiT[k, A-col m].T @ w1[k]
            #                 + sum_k MWgpeT[k, col m].T @ W'B[k]
            wbig = [wr.tile([128, H], BF16, tag=f"wbig{m}", name=f"wbig{m}")
                    for m in range(KX)]
            for g in range(NGA):
                at = load_group(giw_r, gim_r, g * GW, f"gA{g}", wr)
                et = load_group(gew_r, gem_r, g * GW, f"gE{g}", wr)
                for j in range(MG):
                    m = g * MG + j
                    ps = psp.tile([128, H], mybir.dt.float32, tag="ps")
                    n_mm = KG + KE
                    i = 0
                    for k in range(KG):
                        nc.tensor.matmul(ps[:, :],
                                         at[:, k, j * 128:(j + 1) * 128],
                                         w1t[:, k, :],
                                         start=(i == 0), stop=(i == n_mm - 1))
                        i += 1
                    for k in range(KE):
                        nc.tensor.matmul(ps[:, :],
                                         et[:, k, j * 128:(j + 1) * 128],
                                         wpb[k][:, :],
                                         start=(i == 0), stop=(i == n_mm - 1))
                        i += 1
                    nc.vector.tensor_copy(wbig[m][:, :], ps[:, :])

            # ---------- batch loop ----------
            for bt in range(NBT):
                xt = xp.tile([128, KX, BT], BF16, tag="xt")
                nc.gpsimd.dma_start(out=xt[:, :, :],
                                    in_=xt_r[:, :, bt * BT:(bt + 1) * BT])

                h1 = []
                for u in range(KH):
                    ps = psp.tile([128, BT], mybir.dt.float32, tag="ps")
                    for k in range(KX):
                        nc.tensor.matmul(ps[:, :],
                                         wbig[k][:, u * 128:(u + 1) * 128],
                                         xt[:, k, :],
                                         start=(k == 0), stop=(k == KX - 1))
                    t = hp.tile([128, BT], BF16, tag=f"h1_{u}")
                    nc.scalar.activation(t[:, :], ps[:, :], Act.Relu,
                                         bias=bbig[:, u:u + 1])
                    h1.append(t)

                h2 = []
                for m in range(KH):
                    ps = psp.tile([128, BT], mybir.dt.float32, tag="ps")
                    for k in range(KH):
                        nc.tensor.matmul(ps[:, :],
                                         w2t[:, k, m * 128:(m + 1) * 128],
                                         h1[k][:, :],
                                         start=(k == 0), stop=(k == KH - 1))
                    t = hp.tile([128, BT], BF16, tag=f"h2_{m}")
                    nc.scalar.activation(t[:, :], ps[:, :], Act.Relu,
                                         bias=b2c[:, m:m + 1])
                    h2.append(t)

                ps6 = ps6p.tile([A, BT], mybir.dt.float32, tag="ps6")
                for k in range(KH):
                    nc.tensor.matmul(ps6[:, :], w3t[:, k, :], h2[k][:, :],
                                     start=(k == 0), stop=(k == KH - 1))
                osb = op.tile([A, BT], FP32, tag="osb")
                nc.scalar.activation(osb[:, :], ps6[:, :], Act.Relu,
                                     bias=b3c[:, 0:1])
                nc.sync.dma_start(out=o_d[:, bt * BT:(bt + 1) * BT],
                                  in_=osb[:, :])

    nc.finalize()
    return nc


def _get_nc():
    if "nc" not in _CACHE:
        _CACHE["nc"] = _build()
    return _CACHE["nc"]


def _run(inputs, trace=False):
    from concourse.bass_utils import run_bass_kernel_spmd

    nc = _get_nc()
    f32 = np.float32
    asf = lambda a: np.asarray(a, dtype=f32)
    shared = {
        "gpi_wt": np.ascontiguousarray(asf(inputs["gpi_w"]).T),
        "gpi_mask": np.ascontiguousarray(asf(inputs["gpi_mask"])),
        "gpe_wt": np.ascontiguousarray(asf(inputs["gpe_w"]).T),
        "gpe_mask": np.ascontiguousarray(asf(inputs["gpe_mask"])),
        "w1": np.ascontiguousarray(asf(inputs["w1"])),
        "w2": np.ascontiguousarray(asf(inputs["w2"])),
        "w3": np.ascontiguousarray(asf(inputs["w3"])),
        "gpe_b": np.ascontiguousarray(asf(inputs["gpe_b"])),
        "gpi_b": np.ascontiguousarray(asf(inputs["gpi_b"])),
        "b1": np.ascontiguousarray(asf(inputs["b1"])),
        "b2": np.ascontiguousarray(asf(inputs["b2"])),
        "b3": np.ascontiguousarray(asf(inputs["b3"])),
    }
    xT = np.ascontiguousarray(asf(inputs["x"]).T)   # [DIN, B]
    in_maps = [dict(shared, x_t=np.ascontiguousarray(xT[:, c * BS:(c + 1) * BS]))
               for c in range(NCORES)]
    res = run_bass_kernel_spmd(nc, in_maps, list(range(NCORES)), trace=trace)
    out = np.concatenate(
        [np.asarray(res.results[c]["out"]).T for c in range(NCORES)], axis=0)
    return out.astype(f32), res


def kernel(**inputs):
    out, _ = _run(inputs, trace=False)
    return out
